# revision 1
# baseline (speedup 1.0000x reference)
"""Causal self-attention (B=4, T=2048, C=1024, H=16) on 8 TRN2 NeuronCores.

Sharding: core c handles batch b=c//2 and head-half hh=c%2 (8 heads).
Each core computes q/k/v projections for its heads, causal attention, and a
partial output projection (row-parallel w_proj); the host sums the two
partials per batch.

Layout strategy ("transposed attention"): scores are computed as
S^T = K @ Q^T with keys on psum partitions, so softmax needs no on-chip
transposes; exp(S^T) feeds the attn@v matmul directly as the moving operand
(out^T = [V|1]^T-style via a ones column in V giving softmax denominators
for free). All big matmuls run in float32r (full PE rate, ~1e-4 precision).
"""

import sys

sys.path.insert(0, "/opt/trn_rl_repo")

from contextlib import ExitStack

import numpy as np
import ml_dtypes

import concourse.bass as bass
import concourse.tile as tile
from concourse import bacc, mybir
from concourse.bass_utils import run_bass_kernel_spmd

F32 = mybir.dt.float32
F32R = mybir.dt.float32r
BF16 = mybir.dt.bfloat16
AL = mybir.AluOpType
AF = mybir.ActivationFunctionType

B, T, C, H, HD = 4, 2048, 1024, 16, 64
NCORE = 8
HH = H // 2  # heads per core
NP = HH // 2  # head pairs per core
KC = C // 128  # contraction chunks
NT = T // 128  # 128-row time tiles
NQC = T // 512  # 512-query chunks
ROPE_THETA = 10000.0

_CACHE = {}


def _build_module():
    nc = bacc.Bacc("TRN2", target_bir_lowering=False, debug=False)

    xT = nc.dram_tensor("xT", [C, T], F32R, kind="ExternalInput")
    wq = nc.dram_tensor("wq", [C, 512], F32R, kind="ExternalInput")
    wk = nc.dram_tensor("wk", [C, 512], F32R, kind="ExternalInput")
    wv = nc.dram_tensor("wv", [C, 512], F32R, kind="ExternalInput")
    wp = nc.dram_tensor("wp", [512, C], F32R, kind="ExternalInput")
    bqk = nc.dram_tensor("bqk", [2, NP, 128], F32, kind="ExternalInput")
    bv = nc.dram_tensor("bv", [1, 512], F32, kind="ExternalInput")
    bp = nc.dram_tensor("bp", [1, C], F32, kind="ExternalInput")
    cosr = nc.dram_tensor("cosr", [128, T], F32, kind="ExternalInput")
    sinp = nc.dram_tensor("sinp", [128, T], F32, kind="ExternalInput")
    mask = nc.dram_tensor("mask", [128, 2, 128], F32, kind="ExternalInput")
    onesc = nc.dram_tensor("onesc", [128, HH, 1], F32R, kind="ExternalInput")
    y = nc.dram_tensor("y", [T, C], F32, kind="ExternalOutput")

    with tile.TileContext(nc) as tc, ExitStack() as ctx:
        consts = ctx.enter_context(tc.tile_pool(name="consts", bufs=1))
        qkp = ctx.enter_context(tc.tile_pool(name="qkT", bufs=1))
        vpp = ctx.enter_context(tc.tile_pool(name="vpp", bufs=1))

        bqk_sb = consts.tile([128, 2, NP], F32)
        nc.sync.dma_start(out=bqk_sb[:], in_=bqk.rearrange("a p r -> r a p"))
        bv_row = consts.tile([1, 512], F32)
        nc.sync.dma_start(out=bv_row[:], in_=bv[:])
        bvb = consts.tile([128, 512], F32)
        nc.gpsimd.partition_broadcast(bvb[:], bv_row[:])
        bp_row = consts.tile([1, C], F32)
        nc.sync.dma_start(out=bp_row[:], in_=bp[:])
        bpb = consts.tile([128, C], F32)
        nc.gpsimd.partition_broadcast(bpb[:], bp_row[:])
        mask_sb = consts.tile([128, 2, 128], F32)
        nc.sync.dma_start(out=mask_sb[:], in_=mask[:])
        ones_sb = consts.tile([128, HH, 1], F32R)
        nc.sync.dma_start(out=ones_sb[:], in_=onesc[:])

        qT_sb = qkp.tile([128, NP, T], F32R)
        kT_sb = qkp.tile([128, NP, T], F32R)

        # ---- phases 1-2: projections (v natural; q/k transposed + rope) ----
        # v' lives in SBUF for the whole kernel (written directly, no bounce);
        # xT streams from DRAM in chunks so phases fit alongside it.
        vp_sb = vpp.tile([128, NT, HH, 65], F32R)
        with tc.tile_pool(name="ph1", bufs=2) as ph, \
             tc.tile_pool(name="ps12", bufs=6, space="PSUM") as ps:
            with tc.tile_pool(name="wvp", bufs=1) as wvp, \
                 tc.tile_pool(name="xtv", bufs=3) as xtv:
                wv_sb = wvp.tile([128, KC, 512], F32R)
                nc.sync.dma_start(
                    out=wv_sb[:], in_=wv.rearrange("(kc p) n -> p kc n", p=128)
                )
                xTr = xT.rearrange("(kc p) t -> p kc t", p=128)
                for tt in range(NT):
                    xtc = xtv.tile([128, KC, 128], F32R, tag="xtc")
                    nc.sync.dma_start(
                        out=xtc[:], in_=xTr[:, :, tt * 128 : (tt + 1) * 128]
                    )
                    vps = ps.tile([128, 512], F32, tag="qmm")
                    for kc in range(KC):
                        nc.tensor.matmul(
                            vps[:],
                            xtc[:, kc, :],
                            wv_sb[:, kc, :],
                            start=(kc == 0),
                            stop=(kc == KC - 1),
                        )
                    nc.vector.tensor_add(
                        vp_sb[:, tt, :, 0:64],
                        vps.rearrange("p (h d) -> p h d", h=HH),
                        bvb.rearrange("p (h d) -> p h d", h=HH),
                    )
                    nc.vector.tensor_copy(vp_sb[:, tt, :, 64:65], ones_sb[:])

            with tc.tile_pool(name="wqk", bufs=1) as wqk, \
                 tc.tile_pool(name="xtq", bufs=2) as xtq, \
                 tc.tile_pool(name="ropec", bufs=2) as rp, \
                 tc.tile_pool(name="stg", bufs=2) as stg, \
                 tc.tile_pool(name="stg2", bufs=1) as stg2:
                for which, wt, qkout in ((0, wq, qT_sb), (1, wk, kT_sb)):
                    wsb = wqk.tile([128, KC, 512], F32R, tag="wsb")
                    nc.sync.dma_start(
                        out=wsb[:], in_=wt.rearrange("(kc p) m -> p kc m", p=128)
                    )
                    for nq in range(NQC):
                        nk = slice(nq * 512, (nq + 1) * 512)
                        xtc = xtq.tile([128, KC, 512], F32R, tag="xtc2")
                        nc.sync.dma_start(
                            out=xtc[:],
                            in_=xT.rearrange("(kc p) t -> p kc t", p=128)[:, :, nk],
                        )
                        cos_c = rp.tile([128, 512], F32, tag="cos_c")
                        sin_c = rp.tile([128, 512], F32, tag="sin_c")
                        nc.sync.dma_start(out=cos_c[:], in_=cosr[:, nk])
                        nc.sync.dma_start(out=sin_c[:], in_=sinp[:, nk])
                        for p in range(NP):
                            bap = bqk_sb[:, which, p : p + 1]
                            qps = ps.tile([128, 512], F32, tag="qmm")
                            for kc in range(KC):
                                nc.tensor.matmul(
                                    qps[:],
                                    wsb[:, kc, p * 128 : (p + 1) * 128],
                                    xtc[:, kc, :],
                                    start=(kc == 0),
                                    stop=(kc == KC - 1),
                                )
                            # evict psum + bias in one ACT pass (ACT idle here)
                            qsf = stg.tile([128, 512], F32, tag="qsf")
                            nc.scalar.activation(
                                qsf[:], qps[:], AF.Identity, bias=bap
                            )
                            t1 = stg2.tile([128, 512], F32, tag="t1")
                            s1 = stg.tile([128, 512], F32, tag="s1")
                            s2 = stg2.tile([128, 512], F32, tag="s2")
                            nc.vector.tensor_mul(t1[:], qsf[:], cos_c[:])
                            nc.vector.tensor_mul(s1[:], qsf[:], sin_c[:])
                            # rotate-half via 32-row-swap DMAs (sin sign baked)
                            for o0, i0 in ((0, 32), (32, 0), (64, 96), (96, 64)):
                                nc.sync.dma_start(
                                    out=s2[o0 : o0 + 32, :],
                                    in_=s1[i0 : i0 + 32, :],
                                )
                            nc.vector.tensor_add(qkout[:, p, nk], t1[:], s2[:])

        # ---- phases 3-4: attention + output projection ----
        with tc.tile_pool(name="attp", bufs=1) as ap_, \
             tc.tile_pool(name="ptp", bufs=4) as ptp, \
             tc.tile_pool(name="nrm", bufs=2) as nrm, \
             tc.tile_pool(name="pssc", bufs=3, space="PSUM") as pssc, \
             tc.tile_pool(name="pso", bufs=1, space="PSUM") as pso:
            wp_sb = ap_.tile([128, 4, C], F32R)
            nc.sync.dma_start(
                out=wp_sb[:], in_=wp.rearrange("(kc r) n -> r kc n", r=128)
            )
            OT_sb = ap_.tile([128, NP, T], F32R)

            for p in range(NP):
                for j in range(NQC):
                    oA = pso.tile([65, 512], F32, tag="oA")
                    oB = pso.tile([65, 512], F32, tag="oB")
                    nkt = 4 * (j + 1)
                    for kt in range(nkt):
                        i = kt - 4 * j
                        span = 512 if i < 0 else 512 - 128 * i
                        q0 = j * 512 + (512 - span)
                        co = 512 - span
                        sc = pssc.tile([128, 2, 512], F32, tag="sc")
                        for h in range(2):
                            nc.tensor.matmul(
                                sc[:, h, 0:span],
                                kT_sb[
                                    h * 64 : (h + 1) * 64,
                                    p,
                                    kt * 128 : (kt + 1) * 128,
                                ],
                                qT_sb[h * 64 : (h + 1) * 64, p, q0 : q0 + span],
                                start=True,
                                stop=True,
                                tile_position=(h * 64, 0),
                            )
                        pt = ptp.tile([128, 2, 512], F32R, tag="pt")
                        nc.scalar.activation(
                            pt[:, :, 0:span], sc[:, :, 0:span], AF.Exp
                        )
                        if i >= 0:
                            nc.vector.tensor_mul(
                                pt[:, :, 0:128], pt[:, :, 0:128], mask_sb[:]
                            )
                        for h, o in ((0, oA), (1, oB)):
                            nc.tensor.matmul(
                                o[:, co:512],
                                vp_sb[:, kt, p * 2 + h, :],
                                pt[:, h, 0:span],
                                start=(kt == 0),
                                stop=(kt == nkt - 1),
                            )
                    jq = slice(j * 512, (j + 1) * 512)
                    # evict psum fast (frees the single oA/oB slot), then
                    # normalize entirely from SBUF off the PE critical path.
                    # reciprocal cost scales with free size only, so run it on
                    # a DMA-reshaped [64, 8] view of the denominator row.
                    for h, o in ((0, oA), (1, oB)):
                        oc = nrm.tile([65, 512], F32, tag=f"oc{h}")
                        if h == 0:
                            nc.scalar.activation(oc[:], o[:], AF.Identity)
                        else:
                            nc.vector.tensor_copy(oc[:], o[:])
                        dd = nrm.tile([64, 8], F32, tag=f"dd{h}")
                        nc.sync.dma_start(
                            out=dd[:],
                            in_=oc[64:65, :].rearrange("p (a b) -> p a b", a=64),
                        )
                        rr = nrm.tile([64, 8], F32, tag=f"rr{h}")
                        nc.vector.reciprocal(rr[:], dd[:])
                        dr = nrm.tile([1, 512], F32, tag=f"dr{h}")
                        nc.sync.dma_start(
                            out=dr.rearrange("p (a b) -> p a b", a=64), in_=rr[:]
                        )
                        rb = nrm.tile([64, 512], F32, tag=f"rb{h}")
                        nc.gpsimd.partition_broadcast(rb[:], dr[:])
                        nc.vector.tensor_mul(
                            OT_sb[h * 64 : (h + 1) * 64, p, jq], oc[0:64, :], rb[:]
                        )

            for tt in range(NT):
                yps = pssc.tile([128, 2, 512], F32, tag="sc")
                for kc in range(4):
                    for nn in range(2):
                        nc.tensor.matmul(
                            yps[:, nn, :],
                            OT_sb[:, kc, tt * 128 : (tt + 1) * 128],
                            wp_sb[:, kc, nn * 512 : (nn + 1) * 512],
                            start=(kc == 0),
                            stop=(kc == 3),
                        )
                ysb = nrm.tile([128, C], F32, tag="ysb")
                nc.vector.tensor_add(
                    ysb[:], yps.rearrange("p a n -> p (a n)"), bpb[:]
                )
                nc.sync.dma_start(out=y[tt * 128 : (tt + 1) * 128, :], in_=ysb[:])

    nc.compile()
    return nc


def _rope_tables():
    freqs = 1.0 / (ROPE_THETA ** (np.arange(0, HD, 2, dtype=np.float32) / HD))
    ang = np.arange(T, dtype=np.float32)[:, None] * freqs[None, :]  # [T, 32]
    cos = np.cos(ang).T  # [32, T]
    sin = np.sin(ang).T
    cos_rep = np.tile(cos, (4, 1)).astype(np.float32)  # [128, T]
    sgn = np.repeat(np.array([1.0, -1.0, 1.0, -1.0], np.float32), 32)
    sin_pm = (np.tile(sin, (4, 1)) * sgn[:, None]).astype(np.float32)
    return cos_rep, sin_pm


def _prep_inputs(x, w_qkv, b_qkv, w_proj, b_proj):
    cos_rep, sin_pm = _rope_tables()
    km = np.arange(128)
    mask1 = (km[:, None] <= km[None, :]).astype(np.float32)  # keep k <= q
    mask2 = np.stack([mask1, mask1], axis=1)  # [128, 2, 128]
    in_maps = []
    for c in range(NCORE):
        b, hh = c // 2, c % 2
        s = hh * 512
        m = {
            "xT": np.ascontiguousarray(x[b].T),
            "wq": np.ascontiguousarray(w_qkv[:, s : s + 512]) / 8.0,
            "wk": np.ascontiguousarray(w_qkv[:, C + s : C + s + 512]),
            "wv": np.ascontiguousarray(w_qkv[:, 2 * C + s : 2 * C + s + 512]),
            "wp": np.ascontiguousarray(w_proj[s : s + 512, :]),
            "bqk": np.stack(
                [
                    b_qkv[s : s + 512].reshape(NP, 128) / 8.0,
                    b_qkv[C + s : C + s + 512].reshape(NP, 128),
                ]
            ).astype(np.float32),
            "bv": b_qkv[2 * C + s : 2 * C + s + 512][None, :].astype(np.float32),
            "bp": (
                b_proj[None, :].astype(np.float32)
                if hh == 0
                else np.zeros((1, C), np.float32)
            ),
            "onesc": np.ones((128, HH, 1), np.float32),
            "cosr": cos_rep,
            "sinp": sin_pm,
            "mask": mask2,
        }
        in_maps.append(m)
    return in_maps


def _run(x, w_qkv, b_qkv, w_proj, b_proj, trace=False):
    if "nc" not in _CACHE:
        _CACHE["nc"] = _build_module()
    nc = _CACHE["nc"]
    in_maps = _prep_inputs(
        np.asarray(x, np.float32),
        np.asarray(w_qkv, np.float32),
        np.asarray(b_qkv, np.float32),
        np.asarray(w_proj, np.float32),
        np.asarray(b_proj, np.float32),
    )
    res = run_bass_kernel_spmd(nc, in_maps, core_ids=list(range(NCORE)), trace=trace)
    out = np.empty((B, T, C), np.float32)
    for b in range(B):
        out[b] = res.results[2 * b]["y"] + res.results[2 * b + 1]["y"]
    return out, res


def kernel(x, w_qkv, b_qkv, w_proj, b_proj, n_heads=16):
    out, _ = _run(x, w_qkv, b_qkv, w_proj, b_proj, trace=False)
    return out

